# revision 60
# baseline (speedup 1.0000x reference)
"""GQA multi-head attention (B=2, S=2048, H=2048, 16 Q heads / 4 KV heads, RoPE,
causal) on 8 Trainium2 NeuronCores.

Sharding: tensor-parallel over GQA groups (4 groups, each 4 Q heads + 1 KV head)
x data-parallel over batch (2). Core c handles batch b = c // 4, group g = c % 4.
Column-parallel q/k/v projections, row-parallel o_proj; the 4 partial o_proj
outputs per batch (bf16) are summed on the host in fp32.

Per-core kernel (all matmuls bf16 with fp32 PSUM accumulation). The PE matmul
issue stream is the bottleneck (~216ns per N=512 matmul incl. overlapped
LDWEIGHTS), so the structure keeps it dense:
  phase 1: K^T/V^T projections chunk-streamed as X^T chunks land (warmup
           matmuls pad the DMA waits to keep the HAM clock at 2.4 GHz), RoPE
           half-swap via SWDGE sbuf-sbuf DMAs (the sync queue is busy issuing
           input loads), V transposed to key-major on the PE, then Q^T
           projections t-outer so attention column 0 unblocks first.
  phase 2: attention in S^T (keys x queries) layout, key blocks in PAIRS
           sharing a [128, 2, 512] PSUM tile so each ScalarE exp covers 1024
           elems/partition (the 352-cycle ACTIVATE overhead made per-block exp
           scalar-bound).  Causal mask added on the PE on diagonal blocks.
           Row sums via a ones[128,128] matmul - full-array, because M=1
           matmuls run col_grp-restricted and break back-to-back MM
           pipelining (309 vs 216ns), and the result lands already
           partition-broadcast for the 2-op Newton-Raphson reciprocal.
           rs/osum for pair p-1 are emitted after scores+exp of pair p,
           carried ACROSS head boundaries, so every exp hides under ready PE
           work (the PE queue is strict FIFO).
  phase 3: row-parallel o_proj interleaved per q column; its first group's
           h0-h2 matmuls cover the last rs/osum's exp wait.  osum/rowsum/yp
           accumulators share a 4-buffer PSUM ring: 2*2 (scores) + 4 = 8 banks.
"""

import sys

for _p in ("/root/.axon_site", "/root/.axon_site/_ro/trn_rl_repo",
           "/root/.axon_site/_ro/pypackages", "/opt/trn_rl_repo"):
    if _p not in sys.path:
        sys.path.append(_p)

import numpy as np
import ml_dtypes

import concourse.bass as bass
import concourse.tile as tile
import concourse.mybir as mybir
from concourse import bacc
from concourse.bass import ts
from concourse.bass_utils import run_bass_kernel_spmd
from concourse.masks import make_identity, make_upper_triangular
from contextlib import ExitStack

BF16 = ml_dtypes.bfloat16
P = 128
S = 2048
H = 2048
NH = 4          # Q heads per core
DQ = NH * P     # 512
NCH = H // P    # 16 hidden chunks
NKB = S // P    # 16 key blocks
QTS = 512       # query tile (phase 2)
SCALE = 1.0 / float(np.sqrt(128.0))


def build_nc():
    f32 = mybir.dt.float32
    bf16 = mybir.dt.bfloat16
    nc = bacc.Bacc("TRN2", target_bir_lowering=False, debug=False)

    xT = nc.dram_tensor("xT", (H, S), bf16, kind="ExternalInput").ap()
    wqT = nc.dram_tensor("wqT", (H, DQ), bf16, kind="ExternalInput").ap()
    wkT = nc.dram_tensor("wkT", (H, P), bf16, kind="ExternalInput").ap()
    wvT = nc.dram_tensor("wvT", (H, P), bf16, kind="ExternalInput").ap()
    woT = nc.dram_tensor("woT", (DQ, H), bf16, kind="ExternalInput").ap()
    cosT = nc.dram_tensor("cosT", (P, S), bf16, kind="ExternalInput").ap()
    srT = nc.dram_tensor("sinrotT", (P, S), bf16, kind="ExternalInput").ap()
    y = nc.dram_tensor("y", (S, H), bf16, kind="ExternalOutput").ap()
    rss2 = nc.dram_tensor("rss2", (NH * S // QTS, QTS), f32).ap()  # recip scratch

    Exp = mybir.ActivationFunctionType.Exp

    with ExitStack() as ctx:
        tc = ctx.enter_context(tile.TileContext(nc))
        singles = ctx.enter_context(tc.tile_pool(name="singles", bufs=1))

        # Batched input loads: one strided DMA per group-of-chunks (per-DMA
        # issue on the Sync queue costs ~0.6us; 54 small loads would delay the
        # first matmul by ~30us). K/V weights first so the chunk-streamed K/V
        # projections start immediately.
        xT_sb = singles.tile([P, NCH, S], bf16)
        wqT_sb = singles.tile([P, NCH, DQ], bf16)
        wkT_sb = singles.tile([P, NCH, P], bf16)
        wvT_sb = singles.tile([P, NCH, P], bf16)
        xTr = xT.rearrange("(c p) s -> p c s", p=P)
        wqTr = wqT.rearrange("(c p) m -> p c m", p=P)
        cos_sb = singles.tile([P, S], bf16)
        sr_sb = singles.tile([P, S], bf16)
        # Order by first-use time: xT chunks gate the K projection stream
        # (the kernel's startup critical path); cos/sin are first read by K's
        # rope (~40us), wq by the Q projection (~50us), wo by o_proj (~130us).
        nc.sync.dma_start(wkT_sb[:, :, :], wkT.rearrange("(c p) m -> p c m", p=P))
        for c in range(2):  # first chunks individually: K proj streams earliest
            nc.sync.dma_start(xT_sb[:, c, :], xTr[:, c, :])
        nc.sync.dma_start(wvT_sb[:, :, :], wvT.rearrange("(c p) m -> p c m", p=P))
        for c in range(2, 8):
            nc.sync.dma_start(xT_sb[:, c, :], xTr[:, c, :])
        nc.sync.dma_start(cos_sb, cosT)
        nc.sync.dma_start(sr_sb, srT)
        for c in range(8, 12):
            nc.sync.dma_start(xT_sb[:, c, :], xTr[:, c, :])
        nc.sync.dma_start(wqT_sb[:, 0:8, :], wqTr[:, 0:8, :])
        for c in range(12, 16):
            nc.sync.dma_start(xT_sb[:, c, :], xTr[:, c, :])
        nc.sync.dma_start(wqT_sb[:, 8:16, :], wqTr[:, 8:16, :])
        woT_sb = singles.tile([P, NH, H], bf16)
        nc.sync.dma_start(woT_sb[:, :, :], woT.rearrange("(c p) m -> p c m", p=P))

        # Additive causal mask, applied on the PE: a matmul of masknegT.T @ I
        # accumulated into the scores PSUM adds -1e9 where key > query.
        masknegT = singles.tile([P, P], bf16)
        make_upper_triangular(nc, masknegT[:], val=-1e9, diag=False)
        ident = singles.tile([P, P], bf16)
        make_identity(nc, ident[:])
        # Full [128,128] ones: the rowsum matmul then uses all PE column
        # groups (an M=1 matmul runs col_grp-restricted, and switching
        # col_grp between matmuls breaks back-to-back pipelining: 309ns
        # vs 216ns spacing), and every PSUM partition gets the rowsum,
        # which doubles as the partition-broadcast for the normalizer.
        ones = singles.tile([P, P], bf16)
        nc.vector.memset(ones[:], 1.0)

        QT_sb = singles.tile([P, NH, S], bf16)
        KT_sb = singles.tile([P, S], bf16)
        VT_sb = singles.tile([P, S], bf16)
        Vn_sb = singles.tile([P, NKB, P], bf16)
        OT_sb = singles.tile([P, NH, S], bf16)

        # One PSUM pool set for the WHOLE kernel: projections / V-transposes /
        # warmup draw from the attention's rings (same tags), so there is no
        # pool boundary between phase 1 and attention - a pool close/open
        # made the first scores wait ~5us on the last rope's PSUM reads.
        pools = ExitStack()
        sp2 = pools.enter_context(tc.tile_pool(name="sp2", bufs=2, space="PSUM"))
        accp = pools.enter_context(tc.tile_pool(name="accp", bufs=4,
                                                space="PSUM"))
        rp = pools.enter_context(tc.tile_pool(name="rope", bufs=2))
        ptp = pools.enter_context(tc.tile_pool(name="ptp", bufs=3))
        yop = pools.enter_context(tc.tile_pool(name="yop", bufs=4))
        nrm = pools.enter_context(tc.tile_pool(name="nrm", bufs=3))
        ctx.enter_context(pools)

        # ---------------- phase 1: projections + RoPE + V transpose ---------
        if True:
            # PE warmup during the initial DMA window: no-dep matmuls keep the
            # HAM activity monitor busy so real matmuls start at 2.4 GHz.
            # Issued before chunk-gated projection matmuls (strict PE FIFO), so
            # they fill each DMA wait instead of idling into a re-throttle.
            # Lives in an sp-ring slot (unused until attention starts).
            warm = sp2.tile([P, P], f32, tag="sp", name="warm")

            def warmup(n):
                # N=16 matmuls: same HAM-busy coverage per instruction (the
                # ~60-cycle NX dispatch floor dominates) at ~1/8 the MAC
                # energy - the kernel sits at the P0 power envelope.
                for _ in range(n):
                    nc.tensor.matmul(warm[:, 0:16], ident[:], ident[:, 0:16],
                                     start=True, stop=True)

            warmup(128)

            def proj(w_sb, head, tok):
                ps = accp.tile([P, QTS], f32, tag="acc", name="proj")
                for c in range(NCH):
                    nc.tensor.matmul(
                        ps[:, :], w_sb[:, c, ts(head, P)], xT_sb[:, c, ts(tok, QTS)],
                        start=(c == 0), stop=(c == NCH - 1))
                return ps

            def proj_streamed(w_sb, fill=0, ntile=4):
                # chunk-outer: the token tiles accumulate as xT chunks land,
                # so the PE starts ~1 chunk after the first DMA instead of
                # waiting for the full xT load.  Late chunks get more filler
                # (they arrive ~2.4us apart near the end of the load).
                pss = [accp.tile([P, QTS], f32, tag="acc", name=f"pstr{t}")
                       for t in range(ntile)]
                for c in range(NCH):
                    if fill and c >= 2:
                        # Late chunks arrive ~2-3us apart; enough filler keeps
                        # the PE-idle below the 3.4us HAM window so the next
                        # real matmuls don't run at 1.2 GHz.
                        warmup(fill if c < 10 else fill + 28)
                    for t in range(ntile):
                        nc.tensor.matmul(
                            pss[t][:, :], w_sb[:, c, :], xT_sb[:, c, ts(t, QTS)],
                            start=(c == 0), stop=(c == NCH - 1))
                return pss

            def rope(ps, out_region, tok):
                # fp32 intermediates: bf16 here measured SLOWER overall - the
                # denser engine activity tips the chip into the P0 power state
                # and the PE drops 2.4 -> 2.0 GHz (MM gaps 216 -> 259ns).
                qf = rp.tile([P, QTS], f32, tag="qf")
                nc.scalar.copy(qf[:, :], ps[:, :])
                sw = rp.tile([P, QTS], f32, tag="swap", bufs=4)
                # SWDGE (gpsimd) queue: the sync queue is busy issuing the
                # input loads for the first ~40us and would delay K's rope.
                nc.gpsimd.dma_start(sw[0:64, :], qf[64:128, :])
                nc.gpsimd.dma_start(sw[64:128, :], qf[0:64, :])
                t1 = rp.tile([P, QTS], f32, tag="t1", bufs=4)
                nc.vector.tensor_mul(t1[:, :], ps[:, :], cos_sb[:, ts(tok, QTS)])
                t2 = rp.tile([P, QTS], f32, tag="t2")
                nc.vector.tensor_mul(t2[:, :], sw[:, :], sr_sb[:, ts(tok, QTS)])
                nc.vector.tensor_add(out_region, t1[:, :], t2[:, :])

            # K first (chunk-streamed): phase 2 consumes it first.
            # K's ropes are SPLIT: the ps-freeing ops (qf copy, swap DMAs,
            # cos-mul) are emitted for all 4 tiles first, then the V psum
            # evacuations (so they sit early in the DVE FIFO - the first Q
            # projection's acc slot waits on VT tile 0's copy), and the
            # remaining K-rope muls run after.
            kparts = []
            for t, ps in enumerate(proj_streamed(wkT_sb, fill=16)):
                qf = rp.tile([P, QTS], f32, tag="qf")
                nc.scalar.copy(qf[:, :], ps[:, :])
                sw = rp.tile([P, QTS], f32, tag="swap", bufs=4)
                nc.gpsimd.dma_start(sw[0:64, :], qf[64:128, :])
                nc.gpsimd.dma_start(sw[64:128, :], qf[0:64, :])
                t1 = rp.tile([P, QTS], f32, tag="t1", bufs=4)
                nc.vector.tensor_mul(t1[:, :], ps[:, :],
                                     cos_sb[:, ts(t, QTS)])
                kparts.append((sw, t1, t))
            # V proj TILE-SERIAL, no streaming or filler: it runs after K,
            # when all chunks have already landed (K's last matmul waits the
            # final chunk), and tile-serial makes V tile 0 stop ~4us earlier
            # - its evacuation gates the first Q projection's psum slot.
            for t in range(4):
                ps = proj(wvT_sb, 0, t)
                nc.vector.tensor_copy(VT_sb[:, ts(t, QTS)], ps[:, :])
            # Finish K's ropes on GPSIMD (idle; ~2x slower than DVE but KT
            # isn't read until attention ~125us).  Keeping these 8 ops off the
            # DVE lets the V copies and early Q-rope muls flow - the Q-proj
            # psum ring otherwise stalls ~3.3us behind this backlog.
            for sw, t1, t in kparts:
                t2 = rp.tile([P, QTS], f32, tag="t2")
                nc.gpsimd.tensor_mul(t2[:, :], sw[:, :], sr_sb[:, ts(t, QTS)])
                nc.gpsimd.tensor_add(KT_sb[:, ts(t, QTS)], t1[:, :], t2[:, :])
            # t-outer so attention column t=0 (all heads) unblocks first.
            # The V transposes wait on DVE copies queued behind K's ropes;
            # they are only needed by attention (~130us), so defer them
            # behind the t=1 Q projections, past the DVE backlog.
            for t in range(S // QTS):
                for h in range(NH):
                    ps = proj(wqT_sb, h, t)
                    rope(ps, QT_sb[:, h, ts(t, QTS)], t)
                    if t == 1:
                        for b in range(4 * h, 4 * h + 4):
                            tp = accp.tile([P, P], bf16, tag="acc", name="vt")
                            nc.tensor.transpose(tp[:, :], VT_sb[:, ts(b, P)],
                                                ident[:])
                            nc.vector.tensor_copy(Vn_sb[:, b, :], tp[:, :])

        # ------- phase 2 + 3: attention with o_proj interleaved per q column -
        # Key blocks are processed in PAIRS sharing one [128, 2, 512] PSUM
        # scores tile so each exp ACTIVATE covers ~1024 elems/partition: the
        # 352-cycle fixed overhead per ACTIVATE made per-block exp (687ns)
        # slower than the 642ns of PE work per block - attention was
        # scalar-bound.  Paired, exp is ~1147ns vs ~1284ns PE work per pair.
        # rs/osum for pair p-1 are emitted after scores+exp of pair p, so each
        # exp hides under already-ready PE work (PE queue is strict FIFO).
        # osum / rowsum / o_proj accumulators share one 4-buffer PSUM ring
        # ("acc") to fit: 2*2 (sp) + 4 (acc) = 8 banks.
        if True:
            for t in range(S // QTS):
                qs = QTS * t
                nj = 4 * t + 4              # key blocks per q tile this column
                npair = nj // 2

                def norm(h, rs, osum):
                    # 1/rowsum via 2-op Newton-Raphson approx (~2 ULP) straight
                    # off PSUM (already partition-broadcast by the ones matmul).
                    rsc = nrm.tile([P, QTS], f32, tag="rsc")
                    recipB = nrm.tile([P, QTS], f32, tag="recipB")
                    nc.vector.reciprocal_approx_accurate(recipB[:, :], rs[:, :],
                                                         rsc[:, :])
                    nc.vector.tensor_mul(OT_sb[:, h, qs:qs + QTS], osum[:, :],
                                         recipB[:, :])

                def rs_osum(p, pt, h, rs, osum):
                    for jj in range(2):
                        j = 2 * p + jj
                        co = max(0, P * j - qs)
                        nc.tensor.matmul(rs[:, co:QTS], ones[:],
                                         pt[:, jj, co:QTS],
                                         start=(j == 0), stop=(j == nj - 1))
                        nc.tensor.matmul(osum[:, co:QTS], Vn_sb[:, j, :],
                                         pt[:, jj, co:QTS],
                                         start=(j == 0), stop=(j == nj - 1))
                    if p == npair - 1:
                        norm(h, rs, osum)

                # The rs/osum pipeline carries ACROSS head boundaries: head
                # h's last pair is emitted after head h+1's first scores+exp,
                # so its exp wait is covered by ready PE work (was a ~0.8us
                # PE stall at each of the 16 head boundaries).
                pend = None
                for h in range(NH):
                    osum = accp.tile([P, QTS], f32, tag="acc", name="osum")
                    rs = accp.tile([P, QTS], f32, tag="acc", name="rs")
                    for pr in range(npair):
                        sp = sp2.tile([P, 2, QTS], f32, tag="sp")
                        co0 = max(0, P * 2 * pr - qs)
                        for jj in range(2):
                            j = 2 * pr + jj
                            co = max(0, P * j - qs)
                            diag = j >= 4 * t
                            nc.tensor.matmul(
                                sp[:, jj, co:QTS], KT_sb[:, ts(j, P)],
                                QT_sb[:, h, qs + co:qs + QTS],
                                start=True, stop=not diag)
                            if diag:
                                nc.tensor.matmul(sp[:, jj, co:co + P],
                                                 masknegT[:], ident[:],
                                                 start=False, stop=True)
                        pt = ptp.tile([P, 2, QTS], bf16, tag="pt")
                        nc.scalar.activation(pt[:, :, co0:QTS],
                                             sp[:, :, co0:QTS], Exp,
                                             scale=SCALE)
                        if pend is not None:
                            rs_osum(*pend)
                        pend = (pr, pt, h, rs, osum)
                # o_proj for the token blocks whose attention column is done.
                # The first group's h0-h2 matmuls (ready: their norms are
                # long emitted) cover the exp wait of the very last rs/osum.
                for tb in range(4 * t, 4 * t + 4):
                    for ho in range(H // QTS):
                        yp = accp.tile([P, QTS], f32, tag="acc", name="yp")
                        for h in range(NH - 1):
                            nc.tensor.matmul(yp[:, :], OT_sb[:, h, ts(tb, P)],
                                             woT_sb[:, h, ts(ho, QTS)],
                                             start=(h == 0), stop=False)
                        if pend is not None:
                            rs_osum(*pend)
                            pend = None
                        nc.tensor.matmul(yp[:, :], OT_sb[:, NH - 1, ts(tb, P)],
                                         woT_sb[:, NH - 1, ts(ho, QTS)],
                                         start=False, stop=True)
                        yo = yop.tile([P, QTS], bf16, tag="yo")
                        # Final groups of the last column: ScalarE evacuates
                        # (its exps are done; the vector queue is backlogged
                        # ~1.7us at kernel end, delaying the last store).
                        if t == S // QTS - 1 and tb == 4 * t + 3 and ho >= 2:
                            nc.scalar.copy(yo[:, :], yp[:, :])
                        else:
                            nc.vector.tensor_copy(yo[:, :], yp[:, :])
                        nc.sync.dma_start(y[ts(tb, P), ts(ho, QTS)], yo[:, :])

    nc.compile()
    return nc


_NC_CACHE = None


def _get_nc():
    global _NC_CACHE
    if _NC_CACHE is None:
        _NC_CACHE = build_nc()
    return _NC_CACHE


def make_in_maps(hidden_states, position_ids, wq, wk, wv, wo):
    """Host-side sharding: 8 cores = (batch b = core//4) x (GQA group g = core%4)."""
    in_maps = []
    xTs, coss, srs = {}, {}, {}
    for b in range(2):
        xTs[b] = np.ascontiguousarray(hidden_states[b].T).astype(BF16)
        inv = 1.0 / (10000.0 ** (np.arange(0, P, 2, dtype=np.float64) / P))
        invd = np.concatenate([inv, inv]).astype(np.float64)
        fr = invd[:, None] * position_ids[b].astype(np.float64)[None, :]
        coss[b] = np.cos(fr).astype(BF16)
        sr = np.sin(fr).astype(np.float32)
        sr[:64] *= -1.0
        srs[b] = sr.astype(BF16)
    shards = {}
    for g in range(4):
        shards[g] = dict(
            wqT=np.ascontiguousarray(wq[DQ * g:DQ * (g + 1)].T).astype(BF16),
            wkT=np.ascontiguousarray(wk[P * g:P * (g + 1)].T).astype(BF16),
            wvT=np.ascontiguousarray(wv[P * g:P * (g + 1)].T).astype(BF16),
            woT=np.ascontiguousarray(wo[:, DQ * g:DQ * (g + 1)].T).astype(BF16),
        )
    for core in range(8):
        b, g = core // 4, core % 4
        in_maps.append(dict(xT=xTs[b], cosT=coss[b], sinrotT=srs[b], **shards[g]))
    return in_maps


def kernel(hidden_states, position_ids, wq, wk, wv, wo, **run_kwargs):
    nc = _get_nc()
    in_maps = make_in_maps(np.asarray(hidden_states), np.asarray(position_ids),
                           np.asarray(wq), np.asarray(wk), np.asarray(wv),
                           np.asarray(wo))
    res = run_bass_kernel_spmd(nc, in_maps, core_ids=list(range(8)), **run_kwargs)
    out = np.zeros((2, S, H), np.float32)
    for core in range(8):
        out[core // 4] += res.results[core]["y"].astype(np.float32)
    if run_kwargs:
        kernel.last_results = res
    return out



# revision 69
# speedup vs baseline: 1.2479x; 1.2479x over previous
"""GQA multi-head attention (B=2, S=2048, H=2048, 16 Q heads / 4 KV heads, RoPE,
causal) on 8 Trainium2 NeuronCores.

Sharding: tensor-parallel over GQA groups (4 groups, each 4 Q heads + 1 KV head)
x data-parallel over batch (2). Core c handles batch b = c // 4, group g = c % 4.
Column-parallel q/k/v projections, row-parallel o_proj; the 4 partial o_proj
outputs per batch (bf16) are summed on the host in fp32.

Per-core kernel (all matmuls bf16 with fp32 PSUM accumulation). The PE matmul
issue stream is the bottleneck (~216ns per N=512 matmul incl. overlapped
LDWEIGHTS), so the structure keeps it dense:
  phase 1: K^T/V^T projections chunk-streamed as X^T chunks land (warmup
           matmuls pad the DMA waits to keep the HAM clock at 2.4 GHz), RoPE
           half-swap via SWDGE sbuf-sbuf DMAs (the sync queue is busy issuing
           input loads), V transposed to key-major on the PE, then Q^T
           projections t-outer so attention column 0 unblocks first.
  phase 2: attention in S^T (keys x queries) layout, key blocks in PAIRS
           sharing a [128, 2, 512] PSUM tile so each ScalarE exp covers 1024
           elems/partition (the 352-cycle ACTIVATE overhead made per-block exp
           scalar-bound).  Causal mask added on the PE on diagonal blocks.
           Row sums via a ones[128,128] matmul - full-array, because M=1
           matmuls run col_grp-restricted and break back-to-back MM
           pipelining (309 vs 216ns), and the result lands already
           partition-broadcast for the 2-op Newton-Raphson reciprocal.
           rs/osum for pair p-1 are emitted after scores+exp of pair p,
           carried ACROSS head boundaries, so every exp hides under ready PE
           work (the PE queue is strict FIFO).
  phase 3: row-parallel o_proj interleaved per q column; its first group's
           h0-h2 matmuls cover the last rs/osum's exp wait.  osum/rowsum/yp
           accumulators share a 4-buffer PSUM ring: 2*2 (scores) + 4 = 8 banks.
"""

import sys

for _p in ("/root/.axon_site", "/root/.axon_site/_ro/trn_rl_repo",
           "/root/.axon_site/_ro/pypackages", "/opt/trn_rl_repo"):
    if _p not in sys.path:
        sys.path.append(_p)

import numpy as np
import ml_dtypes

import concourse.bass as bass
import concourse.tile as tile
import concourse.mybir as mybir
from concourse import bacc
from concourse.bass import ts
from concourse.bass_utils import run_bass_kernel_spmd
from concourse.masks import make_identity, make_upper_triangular
from contextlib import ExitStack

BF16 = ml_dtypes.bfloat16
P = 128
S = 2048
H = 2048
NH = 4          # Q heads per core
DQ = NH * P     # 512
NCH = H // P    # 16 hidden chunks
NKB = S // P    # 16 key blocks
QTS = 512       # query tile (phase 2)
SCALE = 1.0 / float(np.sqrt(128.0))


def build_nc():
    f32 = mybir.dt.float32
    bf16 = mybir.dt.bfloat16
    nc = bacc.Bacc("TRN2", target_bir_lowering=False, debug=False)

    xT = nc.dram_tensor("xT", (H, S), bf16, kind="ExternalInput").ap()
    wqT = nc.dram_tensor("wqT", (H, DQ), bf16, kind="ExternalInput").ap()
    wkT = nc.dram_tensor("wkT", (H, P), bf16, kind="ExternalInput").ap()
    wvT = nc.dram_tensor("wvT", (H, P), bf16, kind="ExternalInput").ap()
    woT = nc.dram_tensor("woT", (DQ, H), bf16, kind="ExternalInput").ap()
    cosT = nc.dram_tensor("cosT", (P, S), bf16, kind="ExternalInput").ap()
    srT = nc.dram_tensor("sinrotT", (P, S), bf16, kind="ExternalInput").ap()
    y = nc.dram_tensor("y", (S, H), bf16, kind="ExternalOutput").ap()
    rss2 = nc.dram_tensor("rss2", (NH * S // QTS, QTS), f32).ap()  # recip scratch

    Exp = mybir.ActivationFunctionType.Exp

    with ExitStack() as ctx:
        tc = ctx.enter_context(tile.TileContext(nc))
        singles = ctx.enter_context(tc.tile_pool(name="singles", bufs=1))

        # Batched input loads: one strided DMA per group-of-chunks (per-DMA
        # issue on the Sync queue costs ~0.6us; 54 small loads would delay the
        # first matmul by ~30us). K/V weights first so the chunk-streamed K/V
        # projections start immediately.
        xT_sb = singles.tile([P, NCH, S], bf16)
        wqT_sb = singles.tile([P, NCH, DQ], bf16)
        wkT_sb = singles.tile([P, NCH, P], bf16)
        wvT_sb = singles.tile([P, NCH, P], bf16)
        xTr = xT.rearrange("(c p) s -> p c s", p=P)
        wqTr = wqT.rearrange("(c p) m -> p c m", p=P)
        cos_sb = singles.tile([P, S], bf16)
        sr_sb = singles.tile([P, S], bf16)
        # Order by first-use time: xT chunks gate the K projection stream
        # (the kernel's startup critical path); cos/sin are first read by K's
        # rope (~40us), wq by the Q projection (~50us), wo by o_proj (~130us).
        nc.sync.dma_start(wkT_sb[:, :, :], wkT.rearrange("(c p) m -> p c m", p=P))
        for c in range(2):  # first chunks individually: K proj streams earliest
            nc.sync.dma_start(xT_sb[:, c, :], xTr[:, c, :])
        nc.sync.dma_start(wvT_sb[:, :, :], wvT.rearrange("(c p) m -> p c m", p=P))
        for c in range(2, 8):
            nc.sync.dma_start(xT_sb[:, c, :], xTr[:, c, :])
        nc.sync.dma_start(cos_sb, cosT)
        nc.sync.dma_start(sr_sb, srT)
        for c in range(8, 12):
            nc.sync.dma_start(xT_sb[:, c, :], xTr[:, c, :])
        nc.sync.dma_start(wqT_sb[:, 0:8, :], wqTr[:, 0:8, :])
        for c in range(12, 16):
            nc.sync.dma_start(xT_sb[:, c, :], xTr[:, c, :])
        nc.sync.dma_start(wqT_sb[:, 8:16, :], wqTr[:, 8:16, :])
        woT_sb = singles.tile([P, NH, H], bf16)
        nc.sync.dma_start(woT_sb[:, :, :], woT.rearrange("(c p) m -> p c m", p=P))

        # Additive causal mask, applied on the PE: a matmul of masknegT.T @ I
        # accumulated into the scores PSUM adds -1e9 where key > query.
        masknegT = singles.tile([P, P], bf16)
        make_upper_triangular(nc, masknegT[:], val=-1e9, diag=False)
        ident = singles.tile([P, P], bf16)
        make_identity(nc, ident[:])
        # Full [128,128] ones: the rowsum matmul then uses all PE column
        # groups (an M=1 matmul runs col_grp-restricted, and switching
        # col_grp between matmuls breaks back-to-back pipelining: 309ns
        # vs 216ns spacing), and every PSUM partition gets the rowsum,
        # which doubles as the partition-broadcast for the normalizer.
        ones = singles.tile([P, P], bf16)
        nc.vector.memset(ones[:], 1.0)

        QT_sb = singles.tile([P, NH, S], bf16)
        KT_sb = singles.tile([P, S], bf16)
        VT_sb = singles.tile([P, S], bf16)
        Vn_sb = singles.tile([P, NKB, P], bf16)
        OT_sb = singles.tile([P, NH, S], bf16)

        # One PSUM pool set for the WHOLE kernel: projections / V-transposes /
        # warmup draw from the attention's rings (same tags), so there is no
        # pool boundary between phase 1 and attention - a pool close/open
        # made the first scores wait ~5us on the last rope's PSUM reads.
        pools = ExitStack()
        sp2 = pools.enter_context(tc.tile_pool(name="sp2", bufs=2, space="PSUM"))
        accp = pools.enter_context(tc.tile_pool(name="accp", bufs=4,
                                                space="PSUM"))
        rp = pools.enter_context(tc.tile_pool(name="rope", bufs=2))
        ptp = pools.enter_context(tc.tile_pool(name="ptp", bufs=4))
        yop = pools.enter_context(tc.tile_pool(name="yop", bufs=4))
        nrm = pools.enter_context(tc.tile_pool(name="nrm", bufs=4))
        ctx.enter_context(pools)

        # ---------------- phase 1: projections + RoPE + V transpose ---------
        if True:
            # PE warmup during the initial DMA window: no-dep matmuls keep the
            # HAM activity monitor busy so real matmuls start at 2.4 GHz.
            # Issued before chunk-gated projection matmuls (strict PE FIFO), so
            # they fill each DMA wait instead of idling into a re-throttle.
            # Lives in an sp-ring slot (unused until attention starts).
            warm = sp2.tile([P, P], f32, tag="sp", name="warm")

            def warmup(n):
                # N=16 matmuls: same HAM-busy coverage per instruction (the
                # ~60-cycle NX dispatch floor dominates) at ~1/8 the MAC
                # energy - the kernel sits at the P0 power envelope.
                for _ in range(n):
                    nc.tensor.matmul(warm[:, 0:16], ident[:], ident[:, 0:16],
                                     start=True, stop=True)

            warmup(128)

            def proj(w_sb, head, tok):
                ps = accp.tile([P, QTS], f32, tag="acc", name="proj")
                for c in range(NCH):
                    nc.tensor.matmul(
                        ps[:, :], w_sb[:, c, ts(head, P)], xT_sb[:, c, ts(tok, QTS)],
                        start=(c == 0), stop=(c == NCH - 1))
                return ps

            def proj_streamed(w_sb, fill=0, ntile=4):
                # chunk-outer: the token tiles accumulate as xT chunks land,
                # so the PE starts ~1 chunk after the first DMA instead of
                # waiting for the full xT load.  Late chunks get more filler
                # (they arrive ~2.4us apart near the end of the load).
                pss = [accp.tile([P, QTS], f32, tag="acc", name=f"pstr{t}")
                       for t in range(ntile)]
                for c in range(NCH):
                    if fill and c >= 2:
                        # Late chunks arrive ~2-3us apart; enough filler keeps
                        # the PE-idle below the 3.4us HAM window so the next
                        # real matmuls don't run at 1.2 GHz.
                        warmup(fill if c < 10 else fill + 28)
                    for t in range(ntile):
                        nc.tensor.matmul(
                            pss[t][:, :], w_sb[:, c, :], xT_sb[:, c, ts(t, QTS)],
                            start=(c == 0), stop=(c == NCH - 1))
                return pss

            def rope(ps, out_region, tok):
                # fp32 intermediates: bf16 here measured SLOWER overall - the
                # denser engine activity tips the chip into the P0 power state
                # and the PE drops 2.4 -> 2.0 GHz (MM gaps 216 -> 259ns).
                qf = rp.tile([P, QTS], f32, tag="qf")
                nc.scalar.copy(qf[:, :], ps[:, :])
                sw = rp.tile([P, QTS], f32, tag="swap", bufs=4)
                # SWDGE (gpsimd) queue: the sync queue is busy issuing the
                # input loads for the first ~40us and would delay K's rope.
                nc.gpsimd.dma_start(sw[0:64, :], qf[64:128, :])
                nc.gpsimd.dma_start(sw[64:128, :], qf[0:64, :])
                t1 = rp.tile([P, QTS], f32, tag="t1", bufs=4)
                nc.vector.tensor_mul(t1[:, :], ps[:, :], cos_sb[:, ts(tok, QTS)])
                t2 = rp.tile([P, QTS], f32, tag="t2")
                nc.vector.tensor_mul(t2[:, :], sw[:, :], sr_sb[:, ts(tok, QTS)])
                nc.vector.tensor_add(out_region, t1[:, :], t2[:, :])

            # K first (chunk-streamed): phase 2 consumes it first.
            # K's ropes are SPLIT: the ps-freeing ops (qf copy, swap DMAs,
            # cos-mul) are emitted for all 4 tiles first, then the V psum
            # evacuations (so they sit early in the DVE FIFO - the first Q
            # projection's acc slot waits on VT tile 0's copy), and the
            # remaining K-rope muls run after.
            kparts = []
            for t, ps in enumerate(proj_streamed(wkT_sb, fill=16)):
                qf = rp.tile([P, QTS], f32, tag="qf")
                nc.scalar.copy(qf[:, :], ps[:, :])
                sw = rp.tile([P, QTS], f32, tag="swap", bufs=4)
                nc.gpsimd.dma_start(sw[0:64, :], qf[64:128, :])
                nc.gpsimd.dma_start(sw[64:128, :], qf[0:64, :])
                t1 = rp.tile([P, QTS], f32, tag="t1", bufs=4)
                nc.vector.tensor_mul(t1[:, :], ps[:, :],
                                     cos_sb[:, ts(t, QTS)])
                kparts.append((sw, t1, t))
            # V proj TILE-SERIAL, no streaming or filler: it runs after K,
            # when all chunks have already landed (K's last matmul waits the
            # final chunk), and tile-serial makes V tile 0 stop ~4us earlier
            # - its evacuation gates the first Q projection's psum slot.
            for t in range(4):
                ps = proj(wvT_sb, 0, t)
                nc.vector.tensor_copy(VT_sb[:, ts(t, QTS)], ps[:, :])
            # Finish K's ropes on GPSIMD (idle; ~2x slower than DVE but KT
            # isn't read until attention ~125us).  Keeping these 8 ops off the
            # DVE lets the V copies and early Q-rope muls flow - the Q-proj
            # psum ring otherwise stalls ~3.3us behind this backlog.
            for sw, t1, t in kparts:
                t2 = rp.tile([P, QTS], f32, tag="t2")
                nc.gpsimd.tensor_mul(t2[:, :], sw[:, :], sr_sb[:, ts(t, QTS)])
                nc.gpsimd.tensor_add(KT_sb[:, ts(t, QTS)], t1[:, :], t2[:, :])
            # t-outer so attention column t=0 (all heads) unblocks first.
            # The V transposes wait on DVE copies queued behind K's ropes;
            # they are only needed by attention (~130us), so defer them
            # behind the t=1 Q projections, past the DVE backlog.
            for t in range(S // QTS):
                for h in range(NH):
                    ps = proj(wqT_sb, h, t)
                    rope(ps, QT_sb[:, h, ts(t, QTS)], t)
                    if t == 1:
                        for b in range(4 * h, 4 * h + 4):
                            tp = accp.tile([P, P], bf16, tag="acc", name="vt")
                            nc.tensor.transpose(tp[:, :], VT_sb[:, ts(b, P)],
                                                ident[:])
                            nc.vector.tensor_copy(Vn_sb[:, b, :], tp[:, :])

        # ------- phase 2 + 3: attention with o_proj interleaved per q column -
        # Key blocks are processed in PAIRS sharing one [128, 2, 512] PSUM
        # scores tile so each exp ACTIVATE covers ~1024 elems/partition: the
        # 352-cycle fixed overhead per ACTIVATE made per-block exp (687ns)
        # slower than the 642ns of PE work per block - attention was
        # scalar-bound.  Paired, exp is ~1147ns vs ~1284ns PE work per pair.
        # rs/osum for pair p-1 are emitted after scores+exp of pair p, so each
        # exp hides under already-ready PE work (PE queue is strict FIFO).
        # osum / rowsum / o_proj accumulators share one 4-buffer PSUM ring
        # ("acc") to fit: 2*2 (sp) + 4 (acc) = 8 banks.
        if True:
            for t in range(S // QTS):
                qs = QTS * t
                nj = 4 * t + 4              # key blocks per q tile this column
                npair = nj // 2

                def norm(h, rs, osum):
                    # 1/rowsum via the single-op ~51 ULP reciprocal (4e-6
                    # relative - negligible vs the 6e-3 kernel error) straight
                    # off PSUM (already partition-broadcast by the ones matmul).
                    recipB = nrm.tile([P, QTS], f32, tag="recipB")
                    nc.vector.reciprocal_approx_fast(recipB[:, :], rs[:, :])
                    nc.vector.tensor_mul(OT_sb[:, h, qs:qs + QTS], osum[:, :],
                                         recipB[:, :])

                def rs_osum(p, pt, h, rs, osum):
                    for jj in range(2):
                        j = 2 * p + jj
                        co = max(0, P * j - qs)
                        nc.tensor.matmul(rs[:, co:QTS], ones[:],
                                         pt[:, jj, co:QTS],
                                         start=(j == 0), stop=(j == nj - 1))
                        nc.tensor.matmul(osum[:, co:QTS], Vn_sb[:, j, :],
                                         pt[:, jj, co:QTS],
                                         start=(j == 0), stop=(j == nj - 1))
                    if p == npair - 1:
                        norm(h, rs, osum)

                # The rs/osum pipeline carries ACROSS head boundaries at depth
                # TWO: head h's last pair is emitted after head h+1's first
                # two scores+exp pairs, so every exp hides under ~1.7us of
                # ready PE work (depth one left a ~0.5-0.7us stall at each
                # head's last pair).
                pend = []
                for h in range(NH):
                    osum = accp.tile([P, QTS], f32, tag="acc", name="osum")
                    rs = accp.tile([P, QTS], f32, tag="acc", name="rs")
                    for pr in range(npair):
                        sp = sp2.tile([P, 2, QTS], f32, tag="sp")
                        co0 = max(0, P * 2 * pr - qs)
                        for jj in range(2):
                            j = 2 * pr + jj
                            co = max(0, P * j - qs)
                            diag = j >= 4 * t
                            nc.tensor.matmul(
                                sp[:, jj, co:QTS], KT_sb[:, ts(j, P)],
                                QT_sb[:, h, qs + co:qs + QTS],
                                start=True, stop=not diag)
                            if diag:
                                nc.tensor.matmul(sp[:, jj, co:co + P],
                                                 masknegT[:], ident[:],
                                                 start=False, stop=True)
                        pt = ptp.tile([P, 2, QTS], bf16, tag="pt")
                        nc.scalar.activation(pt[:, :, co0:QTS],
                                             sp[:, :, co0:QTS], Exp,
                                             scale=SCALE)
                        if len(pend) == 2:
                            rs_osum(*pend.pop(0))
                        pend.append((pr, pt, h, rs, osum))
                # o_proj for the token blocks whose attention column is done.
                # The first group's h0-h2 matmuls (ready: their norms are
                # long emitted) cover the exp waits of the final rs/osum
                # flushes; BOTH must flush before the h3 matmul, whose norm
                # dependency comes from the last flush (emitting it later in
                # the PE FIFO would deadlock-wait).
                for tb in range(4 * t, 4 * t + 4):
                    for ho in range(H // QTS):
                        yp = accp.tile([P, QTS], f32, tag="acc", name="yp")
                        for h in range(NH - 1):
                            nc.tensor.matmul(yp[:, :], OT_sb[:, h, ts(tb, P)],
                                             woT_sb[:, h, ts(ho, QTS)],
                                             start=(h == 0), stop=False)
                        while pend:
                            rs_osum(*pend.pop(0))
                        nc.tensor.matmul(yp[:, :], OT_sb[:, NH - 1, ts(tb, P)],
                                         woT_sb[:, NH - 1, ts(ho, QTS)],
                                         start=False, stop=True)
                        yo = yop.tile([P, QTS], bf16, tag="yo")
                        # ScalarE evacuates: its exps are done during the
                        # o_proj burst, while the DVE is busy with exactly the
                        # norm chains (recip + OT mul) that gate the h3
                        # matmuls of these same groups.
                        nc.scalar.copy(yo[:, :], yp[:, :])
                        nc.sync.dma_start(y[ts(tb, P), ts(ho, QTS)], yo[:, :])

    nc.compile()
    return nc


_NC_CACHE = None


def _get_nc():
    global _NC_CACHE
    if _NC_CACHE is None:
        _NC_CACHE = build_nc()
    return _NC_CACHE


def make_in_maps(hidden_states, position_ids, wq, wk, wv, wo):
    """Host-side sharding: 8 cores = (batch b = core//4) x (GQA group g = core%4)."""
    in_maps = []
    xTs, coss, srs = {}, {}, {}
    for b in range(2):
        xTs[b] = np.ascontiguousarray(hidden_states[b].T).astype(BF16)
        inv = 1.0 / (10000.0 ** (np.arange(0, P, 2, dtype=np.float64) / P))
        invd = np.concatenate([inv, inv]).astype(np.float64)
        fr = invd[:, None] * position_ids[b].astype(np.float64)[None, :]
        coss[b] = np.cos(fr).astype(BF16)
        sr = np.sin(fr).astype(np.float32)
        sr[:64] *= -1.0
        srs[b] = sr.astype(BF16)
    shards = {}
    for g in range(4):
        shards[g] = dict(
            wqT=np.ascontiguousarray(wq[DQ * g:DQ * (g + 1)].T).astype(BF16),
            wkT=np.ascontiguousarray(wk[P * g:P * (g + 1)].T).astype(BF16),
            wvT=np.ascontiguousarray(wv[P * g:P * (g + 1)].T).astype(BF16),
            woT=np.ascontiguousarray(wo[:, DQ * g:DQ * (g + 1)].T).astype(BF16),
        )
    for core in range(8):
        b, g = core // 4, core % 4
        in_maps.append(dict(xT=xTs[b], cosT=coss[b], sinrotT=srs[b], **shards[g]))
    return in_maps


def kernel(hidden_states, position_ids, wq, wk, wv, wo, **run_kwargs):
    nc = _get_nc()
    in_maps = make_in_maps(np.asarray(hidden_states), np.asarray(position_ids),
                           np.asarray(wq), np.asarray(wk), np.asarray(wv),
                           np.asarray(wo))
    res = run_bass_kernel_spmd(nc, in_maps, core_ids=list(range(8)), **run_kwargs)
    out = np.zeros((2, S, H), np.float32)
    for core in range(8):
        out[core // 4] += res.results[core]["y"].astype(np.float32)
    if run_kwargs:
        kernel.last_results = res
    return out

